# revision 15
# baseline (speedup 1.0000x reference)
"""Causal self-attention (B=2, T=2048, C=1024, NH=16) on 8 trn2 NeuronCores.

Sharding: core c handles batch b = c//4 and head group g = c%4 (4 heads,
256 features). Each core computes q/k/v for its heads, causal attention in
S^T layout (keys on partitions, queries on the free dim), and a partial
output projection  y_heads @ w_proj[head_rows, :].  The host sums the four
partial projections per batch and adds b_proj.

v3: bf16 matmuls (1 cycle/row), X^T via DMA XBAR transpose, QKV emitted
per-512-column chunk chasing the transpose DMAs, attention software-
pipelined one k-tile deep. The two per-head score tiles live in one
2-bank PSUM tile so a single scalar-engine exp covers both heads (halves
the scalar instruction + semaphore count), the diagonal causal mask is one
strided tensor_mul, softmax denominators use reciprocal_approx_fast, and a
warm-up matmul burst keeps the PE HAM clock-gate open during the initial
transpose DMAs.
"""

import os
import sys

import numpy as np

for _p in ("/opt/trn_rl_repo", "/root/.axon_site/_ro/trn_rl_repo"):
    if _p not in sys.path and os.path.isdir(_p):
        sys.path.append(_p)

import concourse.bass as bass  # noqa: E402
import concourse.tile as tile  # noqa: E402
from concourse import bacc, mybir  # noqa: E402
from concourse.bass_utils import run_bass_kernel_spmd  # noqa: E402

P = 128
B, T, C = 2, 2048, 1024
NH, HD = 16, 64
HPC = 4  # heads per core
FPC = HPC * HD  # features per core (256)
QCW = 512  # query-chunk width
F32 = mybir.dt.float32
BF16 = mybir.dt.bfloat16
ADD = mybir.AluOpType.add
MULT = mybir.AluOpType.mult
EXP = mybir.ActivationFunctionType.Exp


def build_nc(t_len: int = T):
    """Build the per-core Bass program (same program on all 8 cores)."""
    nt = t_len // P  # token tiles (16)
    ncb = C // P  # contraction blocks (8)
    nqc = t_len // QCW  # query chunks (4)
    tpq = QCW // P  # token tiles per query chunk (4)
    half = t_len // 2

    nc = bacc.Bacc("TRN2", target_bir_lowering=False, debug=False)

    x_d = nc.dram_tensor("x", [t_len, C], BF16, kind="ExternalInput")
    wq_d = nc.dram_tensor("wq", [C, FPC], BF16, kind="ExternalInput")
    wk_d = nc.dram_tensor("wk", [C, FPC], BF16, kind="ExternalInput")
    wv_d = nc.dram_tensor("wv", [C, FPC], BF16, kind="ExternalInput")
    bqkv_d = nc.dram_tensor("bqkv", [FPC, 3], F32, kind="ExternalInput")
    bv_d = nc.dram_tensor("bv", [1, FPC], BF16, kind="ExternalInput")
    wp_d = nc.dram_tensor("wp", [FPC, C], BF16, kind="ExternalInput")
    triu_d = nc.dram_tensor("triu", [P, P], BF16, kind="ExternalInput")
    out_d = nc.dram_tensor("out", [t_len, C], BF16, kind="ExternalOutput")

    from contextlib import ExitStack

    with tile.TileContext(nc) as tc, ExitStack() as ctx:
        consts = ctx.enter_context(tc.tile_pool(name="consts", bufs=1))
        bigs = ctx.enter_context(tc.tile_pool(name="bigs", bufs=1))
        xts = ctx.enter_context(tc.tile_pool(name="xts", bufs=1))
        qkts = ctx.enter_context(tc.tile_pool(name="qkts", bufs=1))
        yts = ctx.enter_context(tc.tile_pool(name="yts", bufs=1))
        exps = ctx.enter_context(tc.tile_pool(name="exps", bufs=1))
        smalls = ctx.enter_context(tc.tile_pool(name="smalls", bufs=3))
        stage = ctx.enter_context(tc.tile_pool(name="stage", bufs=3))
        psum = ctx.enter_context(tc.tile_pool(name="psum", bufs=2, space="PSUM"))

        # ---- all small weight/const DMAs first (so the big transpose DMAs
        # don't clog the hwdge issue queue ahead of them) ----
        wk_sb = bigs.tile([P, ncb, FPC], BF16, tag="wk")
        wq_sb = bigs.tile([P, ncb, FPC], BF16, tag="wq")
        wv_sb = bigs.tile([P, ncb, FPC], BF16, tag="wv")
        nc.sync.dma_start(out=wk_sb, in_=wk_d.ap().rearrange("(cb p) f -> p cb f", p=P))
        nc.sync.dma_start(out=wq_sb, in_=wq_d.ap().rearrange("(cb p) f -> p cb f", p=P))
        nc.sync.dma_start(out=wv_sb, in_=wv_d.ap().rearrange("(cb p) f -> p cb f", p=P))
        wp_sb = bigs.tile([P, 2, C], BF16, tag="wp")
        nc.sync.dma_start(out=wp_sb, in_=wp_d.ap().rearrange("(fb p) o -> p fb o", p=P))
        bq3 = consts.tile([P, 2, 3], F32)
        nc.sync.dma_start(out=bq3, in_=bqkv_d.ap().rearrange("(b p) c -> p b c", p=P))
        triu2 = consts.tile([P, 2, P], BF16)
        nc.sync.dma_start(out=triu2[:, 0, :], in_=triu_d.ap())
        nc.sync.dma_start(out=triu2[:, 1, :], in_=triu_d.ap())
        bv = consts.tile([1, FPC], BF16)
        nc.sync.dma_start(out=bv, in_=bv_d.ap())
        vrep = consts.tile([P, FPC], BF16)
        nc.gpsimd.dma_start(out=vrep, in_=bv[0:1, None, :].broadcast_to([1, P, FPC]))

        # ---- PE warm-up: keep the HAM activity window busy while the x
        # transposes land, so real matmuls start at 2.4 GHz ----
        for _ in range(40):
            wps = psum.tile([P, P], F32, tag="y", name="warm_ps")
            nc.tensor.matmul(
                wps, wk_sb[:, 0, 0:P], wk_sb[:, 0, 0:P], start=True, stop=True
            )

        # ---- X^T via DMA XBAR transpose, one full [128, T] column block per
        # instruction, issue split across both hwdge queues (sync + scalar)
        # so descriptor generation is not serialized on one engine ----
        xt = [xts.tile([P, t_len], BF16, tag=f"xt{i}", name=f"xt{i}") for i in range(ncb)]
        for cb in range(ncb):
            eng = nc.sync if cb % 2 == 0 else nc.scalar
            eng.dma_start_transpose(
                out=xt[cb], in_=x_d.ap()[:, cb * P : (cb + 1) * P]
            )

        qt = [qkts.tile([P, t_len], BF16, tag=f"qt{i}", name=f"qt{i}") for i in range(2)]
        kt = [qkts.tile([P, t_len], BF16, tag=f"kt{i}", name=f"kt{i}") for i in range(2)]
        # V stored as [P, nt, pair, 130]: per pair, head-A cols 0:65 =
        # [d(64), ones], head-B cols 65:130 = [d(64), ones].
        v_sb = bigs.tile([P, nt, 2, 130], BF16, tag="v")
        nc.vector.memset(v_sb[:, :, :, 64], 1.0)
        nc.vector.memset(v_sb[:, :, :, 129], 1.0)
        yt = [yts.tile([P, t_len], BF16, tag=f"yt{i}", name=f"yt{i}") for i in range(2)]

        # ---------- emission helpers ----------
        def emit_qk_chunk(widx, wsb, dst, pair, qc):
            """One [128, QCW] chunk of Q^T or K^T (8 accumulating matmuls +
            fused bias/scale evacuation)."""
            fs = slice(pair * P, (pair + 1) * P)
            cs = slice(qc * QCW, (qc + 1) * QCW)
            ps = psum.tile([P, QCW], F32, tag="st", name="qk_ps")
            for cb in range(ncb):
                nc.tensor.matmul(
                    ps,
                    wsb[:, cb, fs],
                    xt[cb][:, cs],
                    start=(cb == 0),
                    stop=(cb == ncb - 1),
                )
            bias_ap = bq3[:, pair, widx : widx + 1]
            if widx == 0:  # Q: (q + b) * 1/sqrt(HD)
                nc.vector.tensor_scalar(dst[pair][:, cs], ps, bias_ap, 0.125, ADD, MULT)
            else:
                nc.vector.tensor_scalar_add(dst[pair][:, cs], ps, bias_ap)

        def emit_v_tile(t):
            """V for token tile t (natural layout, bias added via vrep)."""
            ps = psum.tile([P, FPC], F32, tag="st", name="v_ps")
            for cb in range(ncb):
                nc.tensor.matmul(
                    ps,
                    xt[cb][:, t * P : (t + 1) * P],
                    wv_sb[:, cb, :],
                    start=(cb == 0),
                    stop=(cb == ncb - 1),
                )
            nc.vector.tensor_add(
                v_sb[:, t].rearrange("p a (h w) -> p a h w", w=65)[:, :, :, 0:64],
                ps.rearrange("p (a h w) -> p a h w", a=2, w=64),
                vrep.rearrange("p (a h w) -> p a h w", a=2, w=64),
            )

        def emit_proj_t(t):
            """Partial output projection + DMA out for one token tile."""
            ost = stage.tile([P, C], BF16, tag="ost", name="ost")
            for nch in range(2):
                ps = psum.tile([P, QCW], F32, tag="st", name="proj_ps")
                for fb in range(2):
                    nc.tensor.matmul(
                        ps,
                        yt[fb][:, t * P : (t + 1) * P],
                        wp_sb[:, fb, nch * QCW : (nch + 1) * QCW],
                        start=(fb == 0),
                        stop=(fb == 1),
                    )
                nc.vector.tensor_copy(
                    out=ost[:, nch * QCW : (nch + 1) * QCW], in_=ps
                )
            nc.sync.dma_start(out=out_d.ap()[t * P : (t + 1) * P, :], in_=ost)

        # Persistent denominator-packing tiles: head A's sums row lives at
        # partition 0, head B's at partition 32 (engine ops need 32-aligned
        # partition starts), so one reciprocal per group covers both heads.
        ys = smalls.tile([33, QCW], F32, tag="ys", bufs=1, name="ys")
        rec = smalls.tile([33, QCW], F32, tag="rec", bufs=1, name="rec")
        nc.vector.memset(ys, 1.0)

        # Attention pipeline state: at most one un-flushed (S emitted, exp/PV
        # pending) k-tile unit, so S(ki+1) runs on the PE while exp(ki) runs
        # on the scalar engine.
        pending = []
        grp = {}
        chunkq = []  # (qc_tag, thunk) deferred PE work

        def emit_s(pair, qc, ki):
            """Score matmuls for one 128-row k-tile: both heads into one
            2-bank PSUM tile (head A cols 0:QCW, head B cols QCW:2QCW)."""
            cs0 = qc * QCW
            m = ki - tpq * qc
            lo = max(m, 0) * P  # first unmasked query column of this k-tile
            ks = slice(ki * P, (ki + 1) * P)
            stAB = psum.tile([P, 2 * QCW], F32, tag="st2", name="stAB")
            nc.tensor.matmul(
                stAB[:, lo:QCW],
                kt[pair][0:64, ks],
                qt[pair][0:64, cs0 + lo : cs0 + QCW],
                start=True,
                stop=True,
            )
            nc.tensor.matmul(
                stAB[:, QCW + lo :],
                kt[pair][64:P, ks],
                qt[pair][64:P, cs0 + lo : cs0 + QCW],
                start=True,
                stop=True,
                tile_position=(64, 0),
            )
            pending.append((pair, qc, ki, stAB, lo, m))

        def flush_one():
            """exp + mask + PV (+ normalization at group end) for the oldest
            pending k-tile."""
            pair, qc, ki, stAB, lo, m = pending.pop(0)
            nki = tpq * (qc + 1)
            cs = slice(qc * QCW, (qc + 1) * QCW)
            # static per-ki buffer: reuse distance is a whole group, so the
            # scalar engine never waits on (or syncs against) pool rotation
            eAB = exps.tile([P, 2 * QCW], BF16, tag=f"exp{ki}", name="eAB")
            # single exp over both heads; the [QCW : QCW+lo] strip is junk
            # (stale psum) but is never read by the PV matmuls below.
            nc.scalar.activation(eAB[:, lo:], stAB[:, lo:], EXP)
            if m >= 0:  # diagonal 128-block: causal triangle mask, both heads
                ev = eAB.rearrange("p (a w) -> p a w", a=2)[
                    :, :, m * P : (m + 1) * P
                ]
                nc.vector.tensor_mul(ev, ev, triu2)
            if ki == 0:
                grp["yA"] = psum.tile([P, QCW], F32, tag="y", name="yA")
                grp["yB"] = psum.tile([P, QCW], F32, tag="y", name="yB")
            st, sp = ki == 0, ki == nki - 1
            nc.tensor.matmul(
                grp["yA"][0:65, lo:], v_sb[:, ki, pair, 0:65], eAB[:, lo:QCW],
                start=st, stop=sp,
            )
            nc.tensor.matmul(
                grp["yB"][0:65, lo:], v_sb[:, ki, pair, 65:130],
                eAB[:, QCW + lo :],
                start=st, stop=sp,
            )
            if sp:
                # normalize: divide by the denominators the ones-column put
                # in row 64. Both heads' denominator rows are packed into one
                # tile (partitions 0 / 32) so the expensive free-size-priced
                # reciprocal runs once; the muls read y straight from PSUM.
                nc.vector.tensor_copy(out=ys[0:1, :], in_=grp["yA"][64:65, :])
                nc.vector.tensor_copy(out=ys[32:33, :], in_=grp["yB"][64:65, :])
                nc.vector.reciprocal(rec, ys)
                recbA = smalls.tile([64, QCW], F32, tag="recbA", name="recbA")
                recbB = smalls.tile([64, QCW], F32, tag="recbB", name="recbB")
                nc.gpsimd.dma_start(
                    out=recbA, in_=rec[0:1, None, :].broadcast_to([1, 64, QCW])
                )
                nc.gpsimd.dma_start(
                    out=recbB, in_=rec[32:33, None, :].broadcast_to([1, 64, QCW])
                )
                nc.vector.tensor_mul(yt[pair][0:64, cs], grp["yA"][0:64, :], recbA)
                nc.vector.tensor_mul(yt[pair][64:P, cs], grp["yB"][0:64, :], recbB)
                for t in range(qc * tpq, (qc + 1) * tpq):
                    if pair == 1:  # both pairs' yt chunks now ready
                        chunkq.append((qc, lambda tt=t: emit_proj_t(tt)))

        # ---------- main schedule ----------
        # chunkq holds deferred PE work (next-qc QKV chunks, ready proj
        # tiles), pumped one piece per attention unit so the PE always has
        # non-dependent work while the scalar engine runs exp.
        def pump():
            if chunkq:
                chunkq.pop(0)[1]()

        def drain_kqv(qc):
            """Emit any still-queued chunks tagged <= qc (attention of qc
            reads their kt/qt/v_sb output, so program order must have them
            first)."""
            rest = []
            for tag, thunk in chunkq:
                if tag <= qc:
                    thunk()
                else:
                    rest.append((tag, thunk))
            chunkq[:] = rest

        # qc0's QKV emitted directly (attention can't start without it)
        for pair in range(2):
            emit_qk_chunk(1, wk_sb, kt, pair, 0)
        for pair in range(2):
            emit_qk_chunk(0, wq_sb, qt, pair, 0)
        for t in range(tpq):
            emit_v_tile(t)

        for qc in range(nqc):
            if qc + 1 < nqc:  # queue next chunk's QKV for interleaving
                q2 = qc + 1
                for pair in range(2):
                    chunkq.append(
                        (q2, lambda p=pair: emit_qk_chunk(1, wk_sb, kt, p, q2))
                    )
                for pair in range(2):
                    chunkq.append(
                        (q2, lambda p=pair: emit_qk_chunk(0, wq_sb, qt, p, q2))
                    )
                for t in range(q2 * tpq, (q2 + 1) * tpq):
                    chunkq.append((q2, lambda tt=t: emit_v_tile(tt)))
            for pair in range(2):
                for ki in range(tpq * (qc + 1)):
                    emit_s(pair, qc, ki)
                    pump()
                    if len(pending) > 1:
                        flush_one()
            if qc + 1 < nqc:
                drain_kqv(qc + 1)
        while pending:
            flush_one()
        while chunkq:
            pump()

    nc.compile()
    return nc


_NC_CACHE: dict = {}
LAST_RESULT = None


def kernel(x, w_attn, b_attn, w_proj, b_proj):
    global LAST_RESULT
    import ml_dtypes

    bf16 = ml_dtypes.bfloat16
    x = np.asarray(x, np.float32)
    w_attn = np.asarray(w_attn, np.float32)
    b_attn = np.asarray(b_attn, np.float32)
    w_proj = np.asarray(w_proj, np.float32)
    b_proj = np.asarray(b_proj, np.float32)

    if "nc" not in _NC_CACHE:
        _NC_CACHE["nc"] = build_nc(T)
    nc = _NC_CACHE["nc"]

    triu = np.triu(np.ones((P, P), np.float32)).astype(bf16)
    x_bf = x.astype(bf16)

    in_maps = []
    for core in range(8):
        b, g = core // 4, core % 4
        f0 = g * FPC
        bqkv = np.stack(
            [
                b_attn[f0 : f0 + FPC],
                b_attn[C + f0 : C + f0 + FPC],
                b_attn[2 * C + f0 : 2 * C + f0 + FPC],
            ],
            axis=1,
        ).astype(np.float32)
        in_maps.append(
            {
                "x": np.ascontiguousarray(x_bf[b]),
                "wq": np.ascontiguousarray(w_attn[:, f0 : f0 + FPC]).astype(bf16),
                "wk": np.ascontiguousarray(
                    w_attn[:, C + f0 : C + f0 + FPC]
                ).astype(bf16),
                "wv": np.ascontiguousarray(
                    w_attn[:, 2 * C + f0 : 2 * C + f0 + FPC]
                ).astype(bf16),
                "bqkv": np.ascontiguousarray(bqkv),
                "bv": np.ascontiguousarray(
                    b_attn[None, 2 * C + f0 : 2 * C + f0 + FPC]
                ).astype(bf16),
                "wp": np.ascontiguousarray(w_proj[f0 : f0 + FPC, :]).astype(bf16),
                "triu": triu,
            }
        )

    trace = bool(os.environ.get("BASS_TRACE"))
    res = run_bass_kernel_spmd(
        nc,
        in_maps,
        core_ids=list(range(8)),
        trace=trace,
        tmpdir=os.environ.get("KERNEL_TRACE_DIR") or None,
    )
    LAST_RESULT = res

    y = np.empty((B, T, C), np.float32)
    for b in range(B):
        acc = res.results[4 * b]["out"].astype(np.float32)
        for g in range(1, 4):
            acc = acc + res.results[4 * b + g]["out"].astype(np.float32)
        y[b] = acc + b_proj[None, :]
    return y
